# revision 3
# baseline (speedup 1.0000x reference)
"""Multi-head self-attention Trainium2 kernel (8 NeuronCores).

Problem: x[2, 4096, 256] fp32, Wq/Wk/Wv[256, 256]; 8 heads of dk=dv=32.
out[b] = softmax(Q K^T / sqrt(32)) V per head, heads concatenated.

Sharding: 16 (batch, head) pairs over 8 cores -> each core handles one
batch and two adjacent heads. No cross-core communication.

Per-core algorithm (S^T layout, flash-style over key tiles):
  - host passes x[b]^T ([256, 4096]) so feature dim is on partitions
  - QK^T projection: one fused matmul per 512-token chunk produces
    Q^T (rows 0-31) and K^T (rows 32-63) in a [64, 4096] SBUF tile
  - V projection into [128, 32, 33] (token-tiled); column 32 is
    memset to 1.0 -> the att matmul also accumulates the softmax
    denominator for free (M = 33)
  - per 512-query chunk: for each group of 3 key tiles:
      scores S^T[keys, queries] via PE (contraction dk=32, fp32r),
      exp via ACT (scale 1/sqrt(dk) folded into the activation's
      free affine; no max-subtraction needed -- scores are ~N(0,1)),
      att^T += V_aug^T-ish accumulation via PE (contraction keys=128)
  - epilogue: PE-transpose att^T -> [queries, 33], DVE reciprocal of
    column 32 (denominator) and per-partition scale of columns 0-31.

All matmuls use float32r (~1.5e-4 rel err, 1 cycle/row on PE).
"""

import numpy as np

import concourse.bacc as bacc
import concourse.mybir as mybir
import concourse.tile as tile
from concourse.bass_utils import run_bass_kernel_spmd
from concourse.masks import make_identity

BATCH = 2
N = 4096
DIN = 256
NH = 8
DK = 32
DV = 32
HEADS_PER_CORE = 2
N_CORES = 8
SCALE = 1.0 / np.sqrt(DK)

QC = 512  # queries per chunk
N_QC = N // QC  # 8
KT = 128  # keys per tile
N_KT = N // KT  # 32
GROUP = 3  # key tiles per score/exp group (3 PSUM banks)

F32 = mybir.dt.float32
F32R = mybir.dt.float32r


def _groups():
    g = []
    k = 0
    while k < N_KT:
        n = min(GROUP, N_KT - k)
        g.append((k, n))
        k += n
    return g


def build():
    nc = bacc.Bacc("TRN2", target_bir_lowering=False)
    xt_d = nc.dram_tensor("xt", [DIN, N], F32, kind="ExternalInput")
    wqk_d = [
        nc.dram_tensor(f"wqk{i}", [DIN, 2 * DK], F32, kind="ExternalInput")
        for i in range(HEADS_PER_CORE)
    ]
    wv_d = nc.dram_tensor("wv", [DIN, HEADS_PER_CORE * DV], F32, kind="ExternalInput")
    out_d = nc.dram_tensor(
        "out", [N, HEADS_PER_CORE * DV], F32, kind="ExternalOutput"
    )

    with tile.TileContext(nc) as tc:
        with (
            tc.tile_pool(name="persist", bufs=1) as pp,
            tc.tile_pool(name="work", bufs=3) as wp,
            tc.tile_pool(name="ep", bufs=2) as ep,
            tc.tile_pool(name="psum", bufs=1, space="PSUM") as psp,
        ):
            xt_sb = pp.tile([128, 2, N], F32R)
            nc.sync.dma_start(
                xt_sb[:], xt_d.rearrange("(c p) n -> p c n", p=128).bitcast(F32R)
            )
            wqk_sb = []
            for i in range(HEADS_PER_CORE):
                w = pp.tile([128, 2, 2 * DK], F32R, tag=f"wqk{i}")
                nc.sync.dma_start(
                    w[:], wqk_d[i].rearrange("(c p) m -> p c m", p=128).bitcast(F32R)
                )
                wqk_sb.append(w)
            wv_sb = pp.tile([128, 2, HEADS_PER_CORE * DV], F32R)
            nc.sync.dma_start(
                wv_sb[:], wv_d.rearrange("(c p) m -> p c m", p=128).bitcast(F32R)
            )
            ident = pp.tile([128, 128], F32)
            make_identity(nc, ident[:])
            out_sb = pp.tile([128, N // 128, HEADS_PER_CORE * DV], F32)

            for hi in range(HEADS_PER_CORE):
                hc = slice(DV * hi, DV * hi + DV)
                # --- projections ---
                # One fused matmul pair per token chunk produces Q^T in psum
                # rows 0-31 and K^T in rows 32-63; split into two base-0
                # SBUF tiles (matmul requires equal lhsT/rhs base partition).
                qt = pp.tile([32, N], F32R, tag="qt")
                kt = pp.tile([32, N], F32R, tag="kt")
                for c in range(N_QC):
                    ps = psp.tile([128, 1536], F32, tag="scores")
                    cs = slice(QC * c, QC * (c + 1))
                    nc.tensor.matmul(
                        ps[0:64, 0:QC],
                        wqk_sb[hi][:, 0, :],
                        xt_sb[:, 0, cs],
                        start=True,
                        stop=False,
                    )
                    nc.tensor.matmul(
                        ps[0:64, 0:QC],
                        wqk_sb[hi][:, 1, :],
                        xt_sb[:, 1, cs],
                        start=False,
                        stop=True,
                    )
                    nc.vector.tensor_copy(qt[:, cs], ps[0:32, 0:QC])
                    nc.vector.tensor_copy(kt[:, cs], ps[32:64, 0:QC])

                vaug = pp.tile([128, N_KT, DV + 1], F32R, tag="vaug")
                nc.any.memset(vaug[:, :, DV : DV + 1].bitcast(F32), 1.0)
                for t4 in range(N_KT // 4):
                    ps = psp.tile([128, 1536], F32, tag="scores")
                    for j in range(4):
                        t = 4 * t4 + j
                        for c in range(2):
                            nc.tensor.matmul(
                                ps[:, 32 * j : 32 * j + DV],
                                xt_sb[:, c, KT * t : KT * (t + 1)],
                                wv_sb[:, c, hc],
                                start=(c == 0),
                                stop=(c == 1),
                            )
                    nc.vector.tensor_copy(
                        vaug[:, 4 * t4 : 4 * t4 + 4, 0:DV],
                        ps[:, 0:128].rearrange("p (j v) -> p j v", j=4),
                    )

                # --- attention ---
                for qc in range(N_QC):
                    qs = slice(QC * qc, QC * (qc + 1))
                    ps_att = psp.tile([33, 512], F32, tag="att")
                    for g0, gn in _groups():
                        ps_s = psp.tile([128, 1536], F32, tag="scores")
                        for j in range(gn):
                            k = g0 + j
                            nc.tensor.matmul(
                                ps_s[:, QC * j : QC * (j + 1)],
                                kt[:, KT * k : KT * (k + 1)],
                                qt[:, qs],
                                start=True,
                                stop=True,
                            )
                        p_t = wp.tile([128, 1536], F32R, tag="p")
                        nc.scalar.activation(
                            p_t[:, 0 : QC * gn],
                            ps_s[:, 0 : QC * gn],
                            mybir.ActivationFunctionType.Exp,
                            scale=SCALE,
                        )
                        for j in range(gn):
                            k = g0 + j
                            nc.tensor.matmul(
                                ps_att[:, :],
                                vaug[:, k, :],
                                p_t[:, QC * j : QC * (j + 1)],
                                start=(k == 0),
                                stop=(k == N_KT - 1),
                            )
                    # --- epilogue: transpose + normalize ---
                    attT = ep.tile([33, 512], F32, tag="attT")
                    nc.vector.tensor_copy(attT[:], ps_att[:])
                    ps_tr = psp.tile([128, 4, 33], F32, tag="tr")
                    rec = ep.tile([128, 4, 1], F32, tag="rec")
                    for j in range(4):
                        nc.tensor.transpose(
                            ps_tr[:, j, :],
                            attT[:, 128 * j : 128 * (j + 1)],
                            ident[0:33, 0:33],
                        )
                        nc.vector.reciprocal(rec[:, j, :], ps_tr[:, j, DV : DV + 1])
                        nc.vector.tensor_scalar_mul(
                            out_sb[:, 4 * qc + j, hc],
                            ps_tr[:, j, 0:DV],
                            rec[:, j, :],
                        )

            nc.sync.dma_start(
                out_d.rearrange("(t p) c -> p t c", p=128), out_sb[:]
            )
    nc.compile()
    return nc


_NC = None


def _get_nc():
    global _NC
    if _NC is None:
        _NC = build()
    return _NC


def kernel(x, Wq, Wk, Wv):
    x = np.asarray(x, dtype=np.float32)
    Wq = np.asarray(Wq, dtype=np.float32)
    Wk = np.asarray(Wk, dtype=np.float32)
    Wv = np.asarray(Wv, dtype=np.float32)

    xt = [np.ascontiguousarray(x[b].T) for b in range(BATCH)]
    in_maps = []
    for core in range(N_CORES):
        b = core // 4
        h0 = (core % 4) * HEADS_PER_CORE
        m = {"xt": xt[b]}
        for i in range(HEADS_PER_CORE):
            h = h0 + i
            cs = slice(DK * h, DK * (h + 1))
            m[f"wqk{i}"] = np.ascontiguousarray(
                np.concatenate([Wq[:, cs], Wk[:, cs]], axis=1)
            )
        m["wv"] = np.ascontiguousarray(
            Wv[:, DV * h0 : DV * (h0 + HEADS_PER_CORE)]
        )
        in_maps.append(m)

    res = run_bass_kernel_spmd(_get_nc(), in_maps, core_ids=list(range(N_CORES)))
    out = np.empty((BATCH, N, NH * DV), np.float32)
    for core in range(N_CORES):
        b = core // 4
        h0 = (core % 4) * HEADS_PER_CORE
        out[b, :, DV * h0 : DV * (h0 + HEADS_PER_CORE)] = res.results[core]["out"]
    return out


# revision 5
# speedup vs baseline: 1.3283x; 1.3283x over previous
"""Multi-head self-attention Trainium2 kernel (8 NeuronCores).

Problem: x[2, 4096, 256] fp32, Wq/Wk/Wv[256, 256]; 8 heads of dk=dv=32.
out[b] = softmax(Q K^T / sqrt(32)) V per head, heads concatenated.

Sharding: 16 (batch, head) pairs over 8 cores -> each core handles one
batch and two adjacent heads. No cross-core communication.

Per-core algorithm (S^T layout, flash-style over key tiles):
  - host passes x[b]^T ([256, 4096]) so feature dim is on partitions
  - QK^T projection: one fused matmul per 512-token chunk produces
    Q^T (rows 0-31) and K^T (rows 32-63) in a [64, 4096] SBUF tile
  - V projection into [128, 32, 33] (token-tiled); column 32 is
    memset to 1.0 -> the att matmul also accumulates the softmax
    denominator for free (M = 33)
  - per 512-query chunk: for each group of 3 key tiles:
      scores S^T[keys, queries] via PE (contraction dk=32, fp32r),
      exp via ACT (scale 1/sqrt(dk) folded into the activation's
      free affine; no max-subtraction needed -- scores are ~N(0,1)),
      att^T += V_aug^T-ish accumulation via PE (contraction keys=128)
  - epilogue: PE-transpose att^T -> [queries, 33], DVE reciprocal of
    column 32 (denominator) and per-partition scale of columns 0-31.

All matmuls use float32r (~1.5e-4 rel err, 1 cycle/row on PE).
"""

import numpy as np

import concourse.bacc as bacc
import concourse.mybir as mybir
import concourse.tile as tile
from concourse.bass_utils import run_bass_kernel_spmd
from concourse.masks import make_identity

BATCH = 2
N = 4096
DIN = 256
NH = 8
DK = 32
DV = 32
HEADS_PER_CORE = 2
N_CORES = 8
SCALE = 1.0 / np.sqrt(DK)

QC = 512  # queries per chunk
N_QC = N // QC  # 8
KT = 128  # keys per tile
N_KT = N // KT  # 32
GROUP = 3  # key tiles per score/exp group (3 PSUM banks)

F32 = mybir.dt.float32
F32R = mybir.dt.float32r


def _groups():
    g = []
    k = 0
    while k < N_KT:
        n = min(GROUP, N_KT - k)
        g.append((k, n))
        k += n
    return g


def build():
    nc = bacc.Bacc("TRN2", target_bir_lowering=False)
    xt_d = nc.dram_tensor("xt", [DIN, N], F32, kind="ExternalInput")
    wqk_d = [
        nc.dram_tensor(f"wqk{i}", [DIN, 2 * DK], F32, kind="ExternalInput")
        for i in range(HEADS_PER_CORE)
    ]
    wv_d = nc.dram_tensor("wv", [DIN, HEADS_PER_CORE * DV], F32, kind="ExternalInput")
    out_d = nc.dram_tensor(
        "out", [N, HEADS_PER_CORE * DV], F32, kind="ExternalOutput"
    )

    with tile.TileContext(nc) as tc:
        with (
            tc.tile_pool(name="persist", bufs=1) as pp,
            tc.tile_pool(name="work", bufs=3) as wp,
            tc.tile_pool(name="ep", bufs=2) as ep,
            tc.tile_pool(name="psum", bufs=1, space="PSUM") as psp,
        ):
            xt_sb = pp.tile([128, 2, N], F32R)
            nc.sync.dma_start(
                xt_sb[:], xt_d.rearrange("(c p) n -> p c n", p=128).bitcast(F32R)
            )
            wqk_sb = []
            for i in range(HEADS_PER_CORE):
                w = pp.tile([128, 2, 2 * DK], F32R, tag=f"wqk{i}")
                nc.sync.dma_start(
                    w[:], wqk_d[i].rearrange("(c p) m -> p c m", p=128).bitcast(F32R)
                )
                wqk_sb.append(w)
            wv_sb = pp.tile([128, 2, HEADS_PER_CORE * DV], F32R)
            nc.sync.dma_start(
                wv_sb[:], wv_d.rearrange("(c p) m -> p c m", p=128).bitcast(F32R)
            )
            ident = pp.tile([128, 128], F32)
            make_identity(nc, ident[:])
            out_sb = pp.tile([128, N // 128, HEADS_PER_CORE * DV], F32)

            for hi in range(HEADS_PER_CORE):
                hc = slice(DV * hi, DV * hi + DV)
                # --- projections ---
                # One fused matmul pair per token chunk produces Q^T in psum
                # rows 0-31 and K^T in rows 32-63; split into two base-0
                # SBUF tiles (matmul requires equal lhsT/rhs base partition).
                qt = pp.tile([32, N], F32R, tag="qt")
                kt = pp.tile([32, N], F32R, tag="kt")
                for c in range(N_QC):
                    ps = psp.tile([128, 1536], F32, tag="scores")
                    cs = slice(QC * c, QC * (c + 1))
                    nc.tensor.matmul(
                        ps[0:64, 0:QC],
                        wqk_sb[hi][:, 0, :],
                        xt_sb[:, 0, cs],
                        start=True,
                        stop=False,
                    )
                    nc.tensor.matmul(
                        ps[0:64, 0:QC],
                        wqk_sb[hi][:, 1, :],
                        xt_sb[:, 1, cs],
                        start=False,
                        stop=True,
                    )
                    nc.vector.tensor_copy(qt[:, cs], ps[0:32, 0:QC])
                    nc.vector.tensor_copy(kt[:, cs], ps[32:64, 0:QC])

                vaug = pp.tile([128, N_KT, DV + 1], F32R, tag="vaug")
                nc.any.memset(vaug[:, :, DV : DV + 1].bitcast(F32), 1.0)
                for t4 in range(N_KT // 4):
                    ps = psp.tile([128, 1536], F32, tag="scores")
                    for j in range(4):
                        t = 4 * t4 + j
                        for c in range(2):
                            nc.tensor.matmul(
                                ps[:, 32 * j : 32 * j + DV],
                                xt_sb[:, c, KT * t : KT * (t + 1)],
                                wv_sb[:, c, hc],
                                start=(c == 0),
                                stop=(c == 1),
                            )
                    nc.vector.tensor_copy(
                        vaug[:, 4 * t4 : 4 * t4 + 4, 0:DV],
                        ps[:, 0:128].rearrange("p (j v) -> p j v", j=4),
                    )

                # --- attention (software-pipelined by one group so PE's
                # stream is scores(g+1) -> att(g): the att matmuls' wait on
                # ACT's exp output is hidden behind the next score group) ---
                def emit_scores(qc, g0, gn):
                    qs = slice(QC * qc, QC * (qc + 1))
                    ps_s = psp.tile([128, 1536], F32, tag="scores")
                    for j in range(gn):
                        k = g0 + j
                        nc.tensor.matmul(
                            ps_s[:, QC * j : QC * (j + 1)],
                            kt[:, KT * k : KT * (k + 1)],
                            qt[:, qs],
                            start=True,
                            stop=True,
                        )
                    p_t = wp.tile([128, 1536], F32R, tag="p")
                    nc.scalar.activation(
                        p_t[:, 0 : QC * gn],
                        ps_s[:, 0 : QC * gn],
                        mybir.ActivationFunctionType.Exp,
                        scale=SCALE,
                    )
                    return p_t

                def emit_att(ps_att, p_t, g0, gn):
                    for j in range(gn):
                        k = g0 + j
                        nc.tensor.matmul(
                            ps_att[:, :],
                            vaug[:, k, :],
                            p_t[:, QC * j : QC * (j + 1)],
                            start=(k == 0),
                            stop=(k == N_KT - 1),
                        )

                def emit_epilogue(qc, ps_att):
                    attT = ep.tile([33, 512], F32, tag="attT")
                    nc.vector.tensor_copy(attT[:], ps_att[:])
                    ps_tr = psp.tile([128, 4, 33], F32, tag="tr")
                    rec = ep.tile([128, 4, 1], F32, tag="rec")
                    for j in range(4):
                        nc.tensor.transpose(
                            ps_tr[:, j, :],
                            attT[:, 128 * j : 128 * (j + 1)],
                            ident[0:33, 0:33],
                        )
                        nc.vector.reciprocal(rec[:, j, :], ps_tr[:, j, DV : DV + 1])
                        nc.vector.tensor_scalar_mul(
                            out_sb[:, 4 * qc + j, hc],
                            ps_tr[:, j, 0:DV],
                            rec[:, j, :],
                        )

                work = [(qc, g0, gn) for qc in range(N_QC) for g0, gn in _groups()]
                ps_att_by_qc = {}
                pending = None  # (qc, g0, gn, p_t)
                for qc, g0, gn in work:
                    if g0 == 0:
                        ps_att_by_qc[qc] = psp.tile(
                            [33, 512], F32, tag="att", name="ps_att"
                        )
                    p_t = emit_scores(qc, g0, gn)
                    if pending is not None:
                        pqc, pg0, pgn, pp_t = pending
                        emit_att(ps_att_by_qc[pqc], pp_t, pg0, pgn)
                        if pg0 + pgn == N_KT:
                            emit_epilogue(pqc, ps_att_by_qc.pop(pqc))
                    pending = (qc, g0, gn, p_t)
                pqc, pg0, pgn, pp_t = pending
                emit_att(ps_att_by_qc[pqc], pp_t, pg0, pgn)
                emit_epilogue(pqc, ps_att_by_qc.pop(pqc))

            nc.sync.dma_start(
                out_d.rearrange("(t p) c -> p t c", p=128), out_sb[:]
            )
    nc.compile()
    return nc


_NC = None


def _get_nc():
    global _NC
    if _NC is None:
        _NC = build()
    return _NC


def kernel(x, Wq, Wk, Wv):
    x = np.asarray(x, dtype=np.float32)
    Wq = np.asarray(Wq, dtype=np.float32)
    Wk = np.asarray(Wk, dtype=np.float32)
    Wv = np.asarray(Wv, dtype=np.float32)

    xt = [np.ascontiguousarray(x[b].T) for b in range(BATCH)]
    in_maps = []
    for core in range(N_CORES):
        b = core // 4
        h0 = (core % 4) * HEADS_PER_CORE
        m = {"xt": xt[b]}
        for i in range(HEADS_PER_CORE):
            h = h0 + i
            cs = slice(DK * h, DK * (h + 1))
            m[f"wqk{i}"] = np.ascontiguousarray(
                np.concatenate([Wq[:, cs], Wk[:, cs]], axis=1)
            )
        m["wv"] = np.ascontiguousarray(
            Wv[:, DV * h0 : DV * (h0 + HEADS_PER_CORE)]
        )
        in_maps.append(m)

    res = run_bass_kernel_spmd(_get_nc(), in_maps, core_ids=list(range(N_CORES)))
    out = np.empty((BATCH, N, NH * DV), np.float32)
    for core in range(N_CORES):
        b = core // 4
        h0 = (core % 4) * HEADS_PER_CORE
        out[b, :, DV * h0 : DV * (h0 + HEADS_PER_CORE)] = res.results[core]["out"]
    return out


# revision 6
# speedup vs baseline: 1.6125x; 1.2140x over previous
"""Multi-head self-attention Trainium2 kernel (8 NeuronCores).

Problem: x[2, 4096, 256] fp32, Wq/Wk/Wv[256, 256]; 8 heads of dk=dv=32.
out[b] = softmax(Q K^T / sqrt(32)) V per head, heads concatenated.

Sharding: 16 (batch, head) pairs over 8 cores -> each core handles one
batch and two adjacent heads. No cross-core communication.

Per-core algorithm (S^T layout, flash-style over key tiles):
  - host passes x[b]^T ([256, 4096]) so the feature dim is on partitions,
    and Wq/Wk head slices replicated 3x along columns ([256, 96]) so the
    projections produce Q^T/K^T replicated across partition strips
    0-31/32-63/64-95 -- required by the row-packed score matmuls.
  - scores: per 512-query chunk and group of 3 key tiles, 3 CONCURRENT
    K=32 matmuls via tile_position=(32j, 0) (the PE runs separate
    32-row strips in parallel; unpacked K=32 fp32r matmuls run at the
    cold 1.2 GHz rate, ~427ns each, vs ~490ns for a whole pack of 3).
  - exp via ACT reading all 3 PSUM banks in one [128, 512*gn]
    instruction; the 1/sqrt(dk) scale is folded into ACT's free affine.
    No max-subtraction: scores are ~N(0,1) so exp cannot overflow.
  - att^T accumulation: lhsT = V_aug [keys, 33] whose column 32 is 1.0,
    so row 32 of att^T is the softmax denominator for free.
  - epilogue: PE-transpose att^T -> [queries, 33], DVE reciprocal of
    column 32 and per-partition scale of columns 0-31.
  - the whole attention stream is software-pipelined by one group so
    PE's order is scores(g+1) -> att(g); the att matmuls' wait on ACT
    exp output hides behind the next score pack.

All matmuls use float32r (~2.5e-4 final rel err, full PE rate).
"""

import numpy as np

import concourse.bacc as bacc
import concourse.mybir as mybir
import concourse.tile as tile
from concourse.bass_utils import run_bass_kernel_spmd
from concourse.masks import make_identity

BATCH = 2
N = 4096
DIN = 256
NH = 8
DK = 32
DV = 32
HEADS_PER_CORE = 2
N_CORES = 8
SCALE = 1.0 / np.sqrt(DK)

QC = 512  # queries per chunk
N_QC = N // QC  # 8
KT = 128  # keys per tile
N_KT = N // KT  # 32
GROUP = 3  # key tiles per score/exp group (3 PSUM banks, 3 row strips)

F32 = mybir.dt.float32
F32R = mybir.dt.float32r


def _groups():
    g = []
    k = 0
    while k < N_KT:
        n = min(GROUP, N_KT - k)
        g.append((k, n))
        k += n
    return g


def build():
    nc = bacc.Bacc("TRN2", target_bir_lowering=False)
    xt_d = nc.dram_tensor("xt", [DIN, N], F32, kind="ExternalInput")
    # wqr{i}/wkr{i}: per-head Wq/Wk column slice replicated 3x -> [256, 96]
    wqr_d = [
        nc.dram_tensor(f"wqr{i}", [DIN, 3 * DK], F32, kind="ExternalInput")
        for i in range(HEADS_PER_CORE)
    ]
    wkr_d = [
        nc.dram_tensor(f"wkr{i}", [DIN, 3 * DK], F32, kind="ExternalInput")
        for i in range(HEADS_PER_CORE)
    ]
    wv_d = nc.dram_tensor("wv", [DIN, HEADS_PER_CORE * DV], F32, kind="ExternalInput")
    out_d = nc.dram_tensor(
        "out", [N, HEADS_PER_CORE * DV], F32, kind="ExternalOutput"
    )

    with tile.TileContext(nc) as tc:
        with (
            tc.tile_pool(name="persist", bufs=1) as pp,
            tc.tile_pool(name="work", bufs=3) as wp,
            tc.tile_pool(name="ep", bufs=2) as ep,
            tc.tile_pool(name="psum", bufs=1, space="PSUM") as psp,
        ):
            xt_sb = pp.tile([128, 2, N], F32R)
            nc.sync.dma_start(
                xt_sb[:], xt_d.rearrange("(c p) n -> p c n", p=128).bitcast(F32R)
            )
            wqr_sb, wkr_sb = [], []
            for i in range(HEADS_PER_CORE):
                wq = pp.tile([128, 2, 3 * DK], F32R, tag=f"wqr{i}", name=f"wqr{i}")
                nc.sync.dma_start(
                    wq[:], wqr_d[i].rearrange("(c p) m -> p c m", p=128).bitcast(F32R)
                )
                wqr_sb.append(wq)
                wk = pp.tile([128, 2, 3 * DK], F32R, tag=f"wkr{i}", name=f"wkr{i}")
                nc.sync.dma_start(
                    wk[:], wkr_d[i].rearrange("(c p) m -> p c m", p=128).bitcast(F32R)
                )
                wkr_sb.append(wk)
            wv_sb = pp.tile([128, 2, HEADS_PER_CORE * DV], F32R)
            nc.sync.dma_start(
                wv_sb[:], wv_d.rearrange("(c p) m -> p c m", p=128).bitcast(F32R)
            )
            ident = pp.tile([128, 128], F32)
            make_identity(nc, ident[:])
            out_sb = pp.tile([128, N // 128, HEADS_PER_CORE * DV], F32)

            # --- V projection for BOTH heads (shared xt weight loads) ---
            # vaug[hi][:, t, 0:32] = V tile, [:, t, 32] = 1.0
            vaug = []
            for hi in range(HEADS_PER_CORE):
                v = pp.tile([128, N_KT, DV + 1], F32R, tag=f"vaug{hi}", name=f"vaug{hi}")
                nc.any.memset(v[:, :, DV : DV + 1].bitcast(F32), 1.0)
                vaug.append(v)
            for t4 in range(N_KT // 4):
                ps = psp.tile([128, 1536], F32, tag="scores", name="ps_v")
                for j in range(4):
                    t = 4 * t4 + j
                    for c in range(2):
                        nc.tensor.matmul(
                            ps[:, 64 * j : 64 * j + 2 * DV],
                            xt_sb[:, c, KT * t : KT * (t + 1)],
                            wv_sb[:, c, :],
                            start=(c == 0),
                            stop=(c == 1),
                        )
                for hi in range(HEADS_PER_CORE):
                    nc.vector.tensor_copy(
                        vaug[hi][:, 4 * t4 : 4 * t4 + 4, 0:DV],
                        ps[:, 0:256].rearrange("p (j h v) -> p j h v", j=4, h=2)[
                            :, :, hi, :
                        ],
                    )

            for hi in range(HEADS_PER_CORE):
                hc = slice(DV * hi, DV * hi + DV)
                # --- Q/K projections (3x-replicated along partition strips) ---
                qt = pp.tile([96, N], F32R, tag="qt")
                kt = pp.tile([96, N], F32R, tag="kt")
                for c in range(N_QC):
                    cs = slice(QC * c, QC * (c + 1))
                    for dst, w in ((qt, wqr_sb[hi]), (kt, wkr_sb[hi])):
                        ps = psp.tile([128, 1536], F32, tag="scores", name="ps_qk")
                        nc.tensor.matmul(
                            ps[0:96, 0:QC],
                            w[:, 0, :],
                            xt_sb[:, 0, cs],
                            start=True,
                            stop=False,
                        )
                        nc.tensor.matmul(
                            ps[0:96, 0:QC],
                            w[:, 1, :],
                            xt_sb[:, 1, cs],
                            start=False,
                            stop=True,
                        )
                        nc.vector.tensor_copy(dst[:, cs], ps[0:96, 0:QC])

                # --- attention (pipelined by one group) ---
                def emit_scores(qc, g0, gn):
                    qs = slice(QC * qc, QC * (qc + 1))
                    ps_s = psp.tile([128, 1536], F32, tag="scores", name="ps_s")
                    for j in range(gn):
                        k = g0 + j
                        sp = slice(32 * j, 32 * (j + 1))
                        nc.tensor.matmul(
                            ps_s[:, QC * j : QC * (j + 1)],
                            kt[sp, KT * k : KT * (k + 1)],
                            qt[sp, qs],
                            start=True,
                            stop=True,
                            tile_position=(32 * j, 0),
                        )
                    p_t = wp.tile([128, 1536], F32R, tag="p", name="p_t")
                    nc.scalar.activation(
                        p_t[:, 0 : QC * gn],
                        ps_s[:, 0 : QC * gn],
                        mybir.ActivationFunctionType.Exp,
                        scale=SCALE,
                    )
                    return p_t

                def emit_att(ps_att, p_t, g0, gn):
                    for j in range(gn):
                        k = g0 + j
                        nc.tensor.matmul(
                            ps_att[:, :],
                            vaug[hi][:, k, :],
                            p_t[:, QC * j : QC * (j + 1)],
                            start=(k == 0),
                            stop=(k == N_KT - 1),
                        )

                def emit_epilogue(qc, ps_att):
                    attT = ep.tile([33, 512], F32, tag="attT", name="attT")
                    nc.vector.tensor_copy(attT[:], ps_att[:])
                    ps_tr = psp.tile([128, 4, 33], F32, tag="tr", name="ps_tr")
                    rec = ep.tile([128, 4, 1], F32, tag="rec", name="rec")
                    for j in range(4):
                        nc.tensor.transpose(
                            ps_tr[:, j, :],
                            attT[:, 128 * j : 128 * (j + 1)],
                            ident[0:33, 0:33],
                        )
                        nc.vector.reciprocal(rec[:, j, :], ps_tr[:, j, DV : DV + 1])
                        nc.vector.tensor_scalar_mul(
                            out_sb[:, 4 * qc + j, hc],
                            ps_tr[:, j, 0:DV],
                            rec[:, j, :],
                        )

                work = [(qc, g0, gn) for qc in range(N_QC) for g0, gn in _groups()]
                ps_att_by_qc = {}
                pending = None  # (qc, g0, gn, p_t)
                for qc, g0, gn in work:
                    if g0 == 0:
                        ps_att_by_qc[qc] = psp.tile(
                            [33, 512], F32, tag="att", name="ps_att"
                        )
                    p_t = emit_scores(qc, g0, gn)
                    if pending is not None:
                        pqc, pg0, pgn, pp_t = pending
                        emit_att(ps_att_by_qc[pqc], pp_t, pg0, pgn)
                        if pg0 + pgn == N_KT:
                            emit_epilogue(pqc, ps_att_by_qc.pop(pqc))
                    pending = (qc, g0, gn, p_t)
                pqc, pg0, pgn, pp_t = pending
                emit_att(ps_att_by_qc[pqc], pp_t, pg0, pgn)
                emit_epilogue(pqc, ps_att_by_qc.pop(pqc))

            nc.sync.dma_start(
                out_d.rearrange("(t p) c -> p t c", p=128), out_sb[:]
            )
    nc.compile()
    return nc


_NC = None


def _get_nc():
    global _NC
    if _NC is None:
        _NC = build()
    return _NC


def make_in_maps(x, Wq, Wk, Wv):
    x = np.asarray(x, dtype=np.float32)
    Wq = np.asarray(Wq, dtype=np.float32)
    Wk = np.asarray(Wk, dtype=np.float32)
    Wv = np.asarray(Wv, dtype=np.float32)
    xt = [np.ascontiguousarray(x[b].T) for b in range(BATCH)]
    in_maps = []
    for core in range(N_CORES):
        b = core // 4
        h0 = (core % 4) * HEADS_PER_CORE
        m = {"xt": xt[b]}
        for i in range(HEADS_PER_CORE):
            h = h0 + i
            cs = slice(DK * h, DK * (h + 1))
            m[f"wqr{i}"] = np.ascontiguousarray(np.tile(Wq[:, cs], (1, 3)))
            m[f"wkr{i}"] = np.ascontiguousarray(np.tile(Wk[:, cs], (1, 3)))
        m["wv"] = np.ascontiguousarray(
            Wv[:, DV * h0 : DV * (h0 + HEADS_PER_CORE)]
        )
        in_maps.append(m)
    return in_maps


def kernel(x, Wq, Wk, Wv):
    in_maps = make_in_maps(x, Wq, Wk, Wv)
    res = run_bass_kernel_spmd(_get_nc(), in_maps, core_ids=list(range(N_CORES)))
    out = np.empty((BATCH, N, NH * DV), np.float32)
    for core in range(N_CORES):
        b = core // 4
        h0 = (core % 4) * HEADS_PER_CORE
        out[b, :, DV * h0 : DV * (h0 + HEADS_PER_CORE)] = res.results[core]["out"]
    return out


# revision 9
# speedup vs baseline: 1.6270x; 1.0090x over previous
"""Multi-head self-attention Trainium2 kernel (8 NeuronCores).

Problem: x[2, 4096, 256] fp32, Wq/Wk/Wv[256, 256]; 8 heads of dk=dv=32.
out[b] = softmax(Q K^T / sqrt(32)) V per head, heads concatenated.

Sharding: 16 (batch, head) pairs over 8 cores -> each core handles one
batch and two adjacent heads. No cross-core communication.

Per-core algorithm (S^T layout, flash-style over key tiles):
  - host passes x[b]^T ([256, 4096]) so the feature dim is on partitions,
    and Wq/Wk head slices replicated 3x along columns ([256, 96]) so the
    projections produce Q^T/K^T replicated across partition strips
    0-31/32-63/64-95 -- required by the row-packed score matmuls.
  - scores: per 512-query chunk and group of 3 key tiles, 3 CONCURRENT
    K=32 matmuls via tile_position=(32j, 0) (the PE runs separate
    32-row strips in parallel; unpacked K=32 fp32r matmuls run at the
    cold 1.2 GHz rate, ~427ns each, vs ~490ns for a whole pack of 3).
  - exp via ACT reading all 3 PSUM banks in one [128, 512*gn]
    instruction; the 1/sqrt(dk) scale is folded into ACT's free affine.
    No max-subtraction: scores are ~N(0,1) so exp cannot overflow.
  - att^T accumulation: lhsT = V_aug [keys, 33] whose column 32 is 1.0,
    so row 32 of att^T is the softmax denominator for free.
  - epilogue: PE-transpose att^T -> [queries, 33], DVE reciprocal of
    column 32 and per-partition scale of columns 0-31.
  - the whole attention stream is software-pipelined by one group so
    PE's order is scores(g+1) -> att(g); the att matmuls' wait on ACT
    exp output hides behind the next score pack.

All matmuls use float32r (~2.5e-4 final rel err, full PE rate).
"""

import numpy as np

import concourse.bacc as bacc
import concourse.mybir as mybir
import concourse.tile as tile
from concourse.bass_utils import run_bass_kernel_spmd
from concourse.masks import make_identity

BATCH = 2
N = 4096
DIN = 256
NH = 8
DK = 32
DV = 32
HEADS_PER_CORE = 2
N_CORES = 8
SCALE = 1.0 / np.sqrt(DK)

QC = 512  # queries per chunk
N_QC = N // QC  # 8
KT = 128  # keys per tile
N_KT = N // KT  # 32
GROUP = 3  # key tiles per score/exp group (3 PSUM banks, 3 row strips)

F32 = mybir.dt.float32
F32R = mybir.dt.float32r


def _groups():
    g = []
    k = 0
    while k < N_KT:
        n = min(GROUP, N_KT - k)
        g.append((k, n))
        k += n
    return g


def build():
    nc = bacc.Bacc("TRN2", target_bir_lowering=False)
    xt_d = nc.dram_tensor("xt", [DIN, N], F32, kind="ExternalInput")
    # wqr{i}/wkr{i}: per-head Wq/Wk column slice replicated 3x -> [256, 96]
    wqr_d = [
        nc.dram_tensor(f"wqr{i}", [DIN, 3 * DK], F32, kind="ExternalInput")
        for i in range(HEADS_PER_CORE)
    ]
    wkr_d = [
        nc.dram_tensor(f"wkr{i}", [DIN, 3 * DK], F32, kind="ExternalInput")
        for i in range(HEADS_PER_CORE)
    ]
    wv_d = nc.dram_tensor("wv", [DIN, HEADS_PER_CORE * DV], F32, kind="ExternalInput")
    out_d = nc.dram_tensor(
        "out", [N, HEADS_PER_CORE * DV], F32, kind="ExternalOutput"
    )

    with tile.TileContext(nc) as tc:
        with (
            tc.tile_pool(name="persist", bufs=1) as pp,
            tc.tile_pool(name="work", bufs=4) as wp,
            tc.tile_pool(name="ep", bufs=2) as ep,
            tc.tile_pool(name="psum", bufs=1, space="PSUM") as psp,
        ):
            xt_sb = pp.tile([128, 2, N], F32R)
            nc.sync.dma_start(
                xt_sb[:], xt_d.rearrange("(c p) n -> p c n", p=128).bitcast(F32R)
            )
            wqr_sb, wkr_sb = [], []
            for i in range(HEADS_PER_CORE):
                wq = pp.tile([128, 2, 3 * DK], F32R, tag=f"wqr{i}", name=f"wqr{i}")
                nc.sync.dma_start(
                    wq[:], wqr_d[i].rearrange("(c p) m -> p c m", p=128).bitcast(F32R)
                )
                wqr_sb.append(wq)
                wk = pp.tile([128, 2, 3 * DK], F32R, tag=f"wkr{i}", name=f"wkr{i}")
                nc.sync.dma_start(
                    wk[:], wkr_d[i].rearrange("(c p) m -> p c m", p=128).bitcast(F32R)
                )
                wkr_sb.append(wk)
            wv_sb = pp.tile([128, 2, HEADS_PER_CORE * DV], F32R)
            nc.sync.dma_start(
                wv_sb[:], wv_d.rearrange("(c p) m -> p c m", p=128).bitcast(F32R)
            )
            ident = pp.tile([128, 128], F32)
            make_identity(nc, ident[:])
            out_sb = pp.tile([128, N // 128, HEADS_PER_CORE * DV], F32)

            # --- V projection for BOTH heads (shared xt weight loads) ---
            # vaug[hi][:, t, 0:32] = V tile, [:, t, 32] = 1.0
            vaug = []
            for hi in range(HEADS_PER_CORE):
                v = pp.tile([128, N_KT, DV + 1], F32R, tag=f"vaug{hi}", name=f"vaug{hi}")
                nc.any.memset(v[:, :, DV : DV + 1].bitcast(F32), 1.0)
                vaug.append(v)
            for t4 in range(N_KT // 4):
                ps = psp.tile([128, 1536], F32, tag="scores", name="ps_v")
                for j in range(4):
                    t = 4 * t4 + j
                    for c in range(2):
                        nc.tensor.matmul(
                            ps[:, 64 * j : 64 * j + 2 * DV],
                            xt_sb[:, c, KT * t : KT * (t + 1)],
                            wv_sb[:, c, :],
                            start=(c == 0),
                            stop=(c == 1),
                        )
                for hi in range(HEADS_PER_CORE):
                    nc.vector.tensor_copy(
                        vaug[hi][:, 4 * t4 : 4 * t4 + 4, 0:DV],
                        ps[:, 0:256].rearrange("p (j h v) -> p j h v", j=4, h=2)[
                            :, :, hi, :
                        ],
                    )

            for hi in range(HEADS_PER_CORE):
                hc = slice(DV * hi, DV * hi + DV)
                # --- Q/K projections (3x-replicated along partition strips) ---
                qt = pp.tile([96, N], F32R, tag="qt")
                kt = pp.tile([96, N], F32R, tag="kt")
                for c in range(N_QC):
                    cs = slice(QC * c, QC * (c + 1))
                    for dst, w in ((qt, wqr_sb[hi]), (kt, wkr_sb[hi])):
                        ps = psp.tile([128, 1536], F32, tag="scores", name="ps_qk")
                        nc.tensor.matmul(
                            ps[0:96, 0:QC],
                            w[:, 0, :],
                            xt_sb[:, 0, cs],
                            start=True,
                            stop=False,
                        )
                        nc.tensor.matmul(
                            ps[0:96, 0:QC],
                            w[:, 1, :],
                            xt_sb[:, 1, cs],
                            start=False,
                            stop=True,
                        )
                        nc.vector.tensor_copy(dst[:, cs], ps[0:96, 0:QC])

                # --- attention (pipelined by one group) ---
                def emit_scores(qc, g0, gn):
                    qs = slice(QC * qc, QC * (qc + 1))
                    ps_s = psp.tile([128, 1536], F32, tag="scores", name="ps_s")
                    for j in range(gn):
                        k = g0 + j
                        sp = slice(32 * j, 32 * (j + 1))
                        nc.tensor.matmul(
                            ps_s[:, QC * j : QC * (j + 1)],
                            kt[sp, KT * k : KT * (k + 1)],
                            qt[sp, qs],
                            start=True,
                            stop=True,
                            tile_position=(32 * j, 0),
                        )
                    p_t = wp.tile([128, 1536], F32R, tag="p", name="p_t")
                    nc.scalar.activation(
                        p_t[:, 0 : QC * gn],
                        ps_s[:, 0 : QC * gn],
                        mybir.ActivationFunctionType.Exp,
                        scale=SCALE,
                    )
                    return p_t

                def emit_att(ps_att, p_t, g0, gn):
                    for j in range(gn):
                        k = g0 + j
                        nc.tensor.matmul(
                            ps_att[:, :],
                            vaug[hi][:, k, :],
                            p_t[:, QC * j : QC * (j + 1)],
                            start=(k == 0),
                            stop=(k == N_KT - 1),
                        )

                def emit_epilogue(qc, ps_att):
                    attT = ep.tile([33, 512], F32, tag="attT", name="attT")
                    nc.vector.tensor_copy(attT[:], ps_att[:])
                    ps_tr = psp.tile([128, 4, 33], F32, tag="tr", name="ps_tr")
                    rec = ep.tile([128, 4, 1], F32, tag="rec", name="rec")
                    for j in range(4):
                        nc.tensor.transpose(
                            ps_tr[:, j, :],
                            attT[:, 128 * j : 128 * (j + 1)],
                            ident[0:33, 0:33],
                        )
                        nc.vector.reciprocal(rec[:, j, :], ps_tr[:, j, DV : DV + 1])
                        nc.vector.tensor_scalar_mul(
                            out_sb[:, 4 * qc + j, hc],
                            ps_tr[:, j, 0:DV],
                            rec[:, j, :],
                        )

                # Two-group lookahead: ACT exp (1.5us) is longer than a score
                # pack (~0.8us), so depth 1 still stalls PE every group.
                DEPTH = 2
                work = [(qc, g0, gn) for qc in range(N_QC) for g0, gn in _groups()]
                ps_att_by_qc = {}
                pending = []  # [(qc, g0, gn, p_t), ...]

                def drain_one():
                    pqc, pg0, pgn, pp_t = pending.pop(0)
                    if pg0 == 0:
                        ps_att_by_qc[pqc] = psp.tile(
                            [33, 512], F32, tag="att", name="ps_att"
                        )
                    emit_att(ps_att_by_qc[pqc], pp_t, pg0, pgn)
                    if pg0 + pgn == N_KT:
                        emit_epilogue(pqc, ps_att_by_qc.pop(pqc))

                for qc, g0, gn in work:
                    p_t = emit_scores(qc, g0, gn)
                    pending.append((qc, g0, gn, p_t))
                    if len(pending) > DEPTH:
                        drain_one()
                while pending:
                    drain_one()

            nc.sync.dma_start(
                out_d.rearrange("(t p) c -> p t c", p=128), out_sb[:]
            )
    nc.compile()
    return nc


_NC = None


def _get_nc():
    global _NC
    if _NC is None:
        _NC = build()
    return _NC


def make_in_maps(x, Wq, Wk, Wv):
    x = np.asarray(x, dtype=np.float32)
    Wq = np.asarray(Wq, dtype=np.float32)
    Wk = np.asarray(Wk, dtype=np.float32)
    Wv = np.asarray(Wv, dtype=np.float32)
    xt = [np.ascontiguousarray(x[b].T) for b in range(BATCH)]
    in_maps = []
    for core in range(N_CORES):
        b = core // 4
        h0 = (core % 4) * HEADS_PER_CORE
        m = {"xt": xt[b]}
        for i in range(HEADS_PER_CORE):
            h = h0 + i
            cs = slice(DK * h, DK * (h + 1))
            m[f"wqr{i}"] = np.ascontiguousarray(np.tile(Wq[:, cs], (1, 3)))
            m[f"wkr{i}"] = np.ascontiguousarray(np.tile(Wk[:, cs], (1, 3)))
        m["wv"] = np.ascontiguousarray(
            Wv[:, DV * h0 : DV * (h0 + HEADS_PER_CORE)]
        )
        in_maps.append(m)
    return in_maps


def kernel(x, Wq, Wk, Wv):
    in_maps = make_in_maps(x, Wq, Wk, Wv)
    res = run_bass_kernel_spmd(_get_nc(), in_maps, core_ids=list(range(N_CORES)))
    out = np.empty((BATCH, N, NH * DV), np.float32)
    for core in range(N_CORES):
        b = core // 4
        h0 = (core % 4) * HEADS_PER_CORE
        out[b, :, DV * h0 : DV * (h0 + HEADS_PER_CORE)] = res.results[core]["out"]
    return out
